# revision 39
# baseline (speedup 1.0000x reference)
"""Bahdanau-style attention kernel for Trainium2 (8 NeuronCores).

Reference computation (B=32, S=2048, H=1024):
    scores[b,s] = dec[b]@W_dec + enc[b,s]@W_enc + bias      (softmax over s)
    out[b,h]    = sum_s softmax(scores)[b,s] * enc[b,s,h]

Key math facts/measurements this kernel is built around:
  - softmax shift-invariance cancels the dec@W_dec + bias term exactly,
    and normalization is deferred to one final scale by 1/sum(exp).
  - enc converted to bf16 on the host (gate is 2e-2; bf16 end-to-end is
    ~2.5e-3; fp8 measured 2.6e-2 — over the gate).  DMA 16.8 MiB/core.
  - The DVE runs every elementwise op at 1x on this HW (no 16-bit fast
    mode, measured): the fused multiply+accumulate score op (STT) on a
    [128,1024] bf16 tile costs ~1.14us, so the 64 score tiles are ~73us
    of DVE — the critical path.  No other engine can apply the per-column
    w: GpSimd rejects TensorScalar at the ISA check, ScalarE scales are
    per-partition only, the DMA ALU has no mult, and the transposed-PE
    matvec path (NPE>0 below) measured +22us/tile of fabric-wide XBAR
    stalls — all dead ends, kept documented here so they are not re-tried.
  - bf16 also runs the PE weighted-sum matmuls at 1 cycle/row (fp32 is
    4) and halves DMA, so DMA (~47us) and PE (~46us) hide under the DVE.

Measured on TRN2 via axon: ~95us HW exec in the device's fast state
(STT 1.14us), ~112-117us when the shared device runs in its slow state
(every DVE op x1.2); rel err 2.5e-3.  Breakdown: ~6.5us engine boot +
~5.5us ramp (wb + first tile) + 64x1.16us DVE span + ~6us tail.

Sharding: data-parallel over batch, 4 batches per core; W replicated
(broadcast to [P,H] on the host — a pure reshape/copy of the tiny W).
"""

import os
import sys

sys.path.insert(0, "/opt/trn_rl_repo")

import numpy as np
import ml_dtypes

import concourse.bass as bass
import concourse.tile as tile
from concourse import bacc, mybir
from concourse.bass_utils import run_bass_kernel_spmd

B, S, H = 32, 2048, 1024
NCORES = 8
BL = B // NCORES          # 4 batches per core
P = 128                   # SBUF partitions
T = S // P                # 16 s-tiles per batch
NB = H // P               # 8 h-blocks per tile
F32 = mybir.dt.float32
BF16 = mybir.dt.bfloat16
FP8 = mybir.dt.float8e4

EXP_G = int(os.environ.get("EXP_G", "4"))    # exp group width (columns)
ENC_BUFS = int(os.environ.get("ENC_BUFS", "40"))
# Transposed-PE score tiles per batch.  Leave at 0: each XBAR transpose
# stalls the whole 16-engine DMA fabric (~1us) and measured +22us/tile of
# end-to-end serialization — the PE score path loses to the DVE's 1.14us.
NPE = int(os.environ.get("NPE", "0"))

LAST_RESULTS = None       # test harness introspection


def _build_bass():
    nc = bacc.Bacc("TRN2", target_bir_lowering=False, debug=False)

    enc = nc.dram_tensor("enc", [BL, S, H], BF16, kind="ExternalInput").ap()
    # wenc arrives pre-broadcast to [P, H] from the host (W is tiny).
    wenc = nc.dram_tensor("wenc", [P, H], BF16, kind="ExternalInput").ap()
    wcol = nc.dram_tensor("wcol", [P, NB], BF16, kind="ExternalInput").ap()
    out = nc.dram_tensor("out", [BL, H], F32, kind="ExternalOutput").ap()

    # PE-path tiles: the last NPE columns of each batch, so the DVE exp
    # groups stay contiguous over columns 0..T-NPE-1.
    pe_set = set(range(T - NPE, T)) if NPE > 0 else set()
    ndve = T - NPE

    with tile.TileContext(nc) as tc:
        from contextlib import ExitStack

        with ExitStack() as ctx:
            wpool = ctx.enter_context(tc.tile_pool(name="wpool", bufs=1))
            encp = ctx.enter_context(tc.tile_pool(name="encp", bufs=ENC_BUFS))
            encTp = ctx.enter_context(tc.tile_pool(name="encTp", bufs=2 * max(NPE, 1)))
            scr = ctx.enter_context(tc.tile_pool(name="scr", bufs=4))
            sp = ctx.enter_context(tc.tile_pool(name="sp", bufs=2))
            # PSUM is 8 banks: psp = ps (2 banks) x 2 bufs; pstp = the batch's
            # 3 matvec chains share one [1, 3*128] bank x 2 bufs; psb per-buf
            # = ecol (1 bank) + pt (1 bank), 1 buf.
            psp = ctx.enter_context(tc.tile_pool(name="psp", bufs=2, space="PSUM"))
            pstp = ctx.enter_context(tc.tile_pool(name="pstp", bufs=2, space="PSUM"))
            psb = ctx.enter_context(tc.tile_pool(name="psb", bufs=1, space="PSUM"))

            # wb load on the scalar engine's HWDGE queue so it is not stuck
            # behind the first batch's enc DMAs on the sync queues; split by
            # PARTITIONS (32 full-width descriptors each) so four queues
            # carry it and it lands ~5us in — the first score op gates on it.
            wb = wpool.tile([P, H], BF16, name="wb")
            for i in range(4):
                nc.scalar.dma_start(
                    wb[32 * i : 32 * (i + 1), :], wenc[32 * i : 32 * (i + 1), :]
                )
            if NPE > 0:
                # NOTE: [128, 8] = 128 sixteen-byte descriptors — keep this
                # tiny DMA off the enc path and NEVER emit it when unused.
                wcolT = wpool.tile([P, NB], BF16, name="wcolT")
                nc.scalar.dma_start(wcolT[:], wcol[:])
            ones = wpool.tile([P, 1], F32, name="ones")
            nc.vector.memset(ones[:], 1.0)
            one1 = wpool.tile([1, 1], BF16, name="one1")
            nc.vector.memset(one1[:], 1.0)

            for b in range(BL):
                enc_b = enc[b].rearrange("(t p) h -> t p h", p=P)  # [T,P,H] view

                tiles = {}
                encT = {}
                for t in range(T):
                    et = encp.tile([P, H], BF16, name=f"enc_{b}_{t}", tag="enc")
                    if b == 0 and t == 0:
                        # Ramp: split tile 0 over 4 queues so scoring starts
                        # ~3x earlier (one DMA binds its descriptors to a
                        # single queue).
                        for i in range(4):
                            nc.sync.dma_start(
                                et[32 * i : 32 * (i + 1), :],
                                enc_b[t][32 * i : 32 * (i + 1), :],
                            )
                    else:
                        nc.sync.dma_start(et[:], enc_b[t])
                    tiles[t] = et
                    if t >= 4 and (t - 4) % 5 == 0 and (t - 4) // 5 < NPE:
                        # Whole-tile XBAR transpose (block-major [p, k, s]) on
                        # the scalar HWDGE queue.  A transpose spreads its
                        # descriptors over ALL 16 DMA engines (~1us fabric-wide
                        # stall), so emit at most one per ~5 enc tiles.
                        tp = sorted(pe_set)[(t - 4) // 5]
                        eT = encTp.tile(
                            [P, H], BF16, name=f"encT_{b}_{tp}", tag="encT"
                        )
                        nc.scalar.dma_start(
                            eT[:].rearrange("p (k s) -> p k s", k=NB),
                            enc_b[tp],
                            transpose=True,
                        )
                        encT[tp] = eT

                e = sp.tile([P, T], BF16, name=f"e_{b}", tag="e")
                ps = psp.tile([1, H], F32, name=f"ps_{b}", tag="ps")

                # --- PE-path matvec chains (PE only; no Scalar coupling) ---
                pst_all = (
                    pstp.tile([1, NPE * P], F32, name=f"pst_{b}", tag="pst")
                    if NPE > 0
                    else None
                )
                for i, t in enumerate(sorted(pe_set)):
                    pst = pst_all[:, i * P : (i + 1) * P]
                    for k in range(NB):
                        nc.tensor.matmul(
                            pst,
                            lhsT=wcolT[:, k : k + 1],
                            rhs=encT[t][:, k * P : (k + 1) * P],
                            start=(k == 0),
                            stop=(k == NB - 1),
                            skip_group_check=True,
                        )

                # --- DVE scores + exp groups + ws ---
                escore = sp.tile([P, T], F32, name=f"escore_{b}", tag="escore")
                for t in range(ndve):
                    stt_out = scr.tile([P, H], FP8, name=f"stt_{b}_{t}", tag="stt")
                    nc.vector.scalar_tensor_tensor(
                        out=stt_out[:],
                        in0=tiles[t][:],
                        scalar=1.0,
                        in1=wb[:],
                        op0=mybir.AluOpType.mult,
                        op1=mybir.AluOpType.mult,
                        accum_out=escore[:, t : t + 1],
                    )

                groups = []
                lo = 0
                while lo < ndve:
                    hi = min(lo + EXP_G, ndve)
                    groups.append((lo, hi))
                    lo = hi
                if b == BL - 1 and groups:
                    # last batch: per-column tail
                    lo, hi = groups.pop()
                    groups += [(t, t + 1) for t in range(lo, hi)]
                for lo, hi in groups:
                    nc.scalar.activation(
                        e[:, lo:hi], escore[:, lo:hi],
                        mybir.ActivationFunctionType.Exp,
                    )
                    for t in range(lo, hi):
                        for h0 in (0, 512):
                            nc.tensor.matmul(
                                ps[:, h0 : h0 + 512],
                                lhsT=e[:, t : t + 1],
                                rhs=tiles[t][:, h0 : h0 + 512],
                                start=(t == 0),
                                stop=(not pe_set) and (t == ndve - 1),
                            )

                # --- PE-path e-columns (Scalar work sits BEHIND the exp
                # groups so it never delays the DVE-side ws chain) ---
                for i, t in enumerate(sorted(pe_set)):
                    erow = sp.tile([1, P], BF16, name=f"erow_{b}_{t}", tag="erow")
                    nc.scalar.activation(
                        erow[:], pst_all[:, i * P : (i + 1) * P],
                        mybir.ActivationFunctionType.Exp,
                    )
                    ecol_ps = psb.tile([P, 1], F32, name=f"ecol_{b}_{t}", tag="ecol")
                    nc.tensor.matmul(
                        ecol_ps[:], lhsT=erow[:], rhs=one1[:], start=True, stop=True
                    )
                    nc.scalar.copy(e[:, t : t + 1], ecol_ps[:])

                # PE-path ws matmuls close the chain.
                last_pe = max(pe_set) if pe_set else None
                for t in sorted(pe_set):
                    for h0 in (0, 512):
                        nc.tensor.matmul(
                            ps[:, h0 : h0 + 512],
                            lhsT=e[:, t : t + 1],
                            rhs=tiles[t][:, h0 : h0 + 512],
                            start=False,
                            stop=(t == last_pe),
                        )

                esum = sp.tile([P, 1], F32, name=f"esum_{b}", tag="esum")
                nc.vector.tensor_reduce(
                    esum[:], e[:], axis=mybir.AxisListType.X,
                    op=mybir.AluOpType.add,
                )
                pt = psb.tile([1, 1], F32, name=f"pt_{b}", tag="pt")
                nc.tensor.matmul(pt[:], lhsT=ones[:], rhs=esum[:], start=True, stop=True)
                rtot = sp.tile([1, 1], F32, name=f"rtot_{b}", tag="rtot")
                nc.vector.reciprocal(rtot[:], pt[:])
                ob = sp.tile([1, H], F32, name=f"ob_{b}", tag="ob")
                for h0 in (0, 512):
                    nc.scalar.mul(ob[:, h0 : h0 + 512], ps[:, h0 : h0 + 512], rtot[:])
                    nc.sync.dma_start(out[b : b + 1, h0 : h0 + 512], ob[:, h0 : h0 + 512])

    nc.compile()
    return nc


_NC_CACHE = None


def kernel(decoder_hidden, encoder_hidden_outputs, W, b):
    global _NC_CACHE, LAST_RESULTS
    enc_full = np.ascontiguousarray(
        np.asarray(encoder_hidden_outputs, dtype=np.float32).astype(ml_dtypes.bfloat16)
    )
    w_enc16 = np.asarray(W, dtype=np.float32)[H:, 0].astype(ml_dtypes.bfloat16)
    w_bcast = np.ascontiguousarray(np.broadcast_to(w_enc16, (P, H)))
    w_col = np.ascontiguousarray(w_enc16.reshape(NB, P).T)

    if _NC_CACHE is None:
        _NC_CACHE = _build_bass()
    nc = _NC_CACHE

    in_maps = [
        {"enc": enc_full[i * BL : (i + 1) * BL], "wenc": w_bcast, "wcol": w_col}
        for i in range(NCORES)
    ]
    res = run_bass_kernel_spmd(
        nc,
        in_maps,
        core_ids=list(range(NCORES)),
        trace=bool(int(os.environ.get("KERNEL_TRACE", "0"))),
    )
    LAST_RESULTS = res
    out = np.concatenate([res.results[i]["out"] for i in range(NCORES)], axis=0)
    return out.astype(np.float32)


# revision 41
# speedup vs baseline: 1.0352x; 1.0352x over previous
"""Bahdanau-style attention kernel for Trainium2 (8 NeuronCores).

Reference computation (B=32, S=2048, H=1024):
    scores[b,s] = dec[b]@W_dec + enc[b,s]@W_enc + bias      (softmax over s)
    out[b,h]    = sum_s softmax(scores)[b,s] * enc[b,s,h]

Key math facts/measurements this kernel is built around:
  - softmax shift-invariance cancels the dec@W_dec + bias term exactly,
    and normalization is deferred to one final scale by 1/sum(exp).
  - enc converted to bf16 on the host (gate is 2e-2; bf16 end-to-end is
    ~2.5e-3; fp8 measured 2.6e-2 — over the gate).  DMA 16.8 MiB/core.
  - The DVE runs every elementwise op at 1x on this HW (no 16-bit fast
    mode, measured): the fused multiply+accumulate score op (STT) on a
    [128,1024] bf16 tile costs ~1.14us, so the 64 score tiles are ~73us
    of DVE — the critical path.  No other engine can apply the per-column
    w: GpSimd rejects TensorScalar at the ISA check, ScalarE scales are
    per-partition only, the DMA ALU has no mult, and the transposed-PE
    matvec path (NPE>0 below) measured +22us/tile of fabric-wide XBAR
    stalls — all dead ends, kept documented here so they are not re-tried.
  - bf16 also runs the PE weighted-sum matmuls at 1 cycle/row (fp32 is
    4) and halves DMA, so DMA (~47us) and PE (~46us) hide under the DVE.

Measured on TRN2 via axon: ~95us HW exec in the device's fast state
(STT 1.14us), ~112-117us when the shared device runs in its slow state
(every DVE op x1.2); rel err 2.5e-3.  Breakdown: ~6.5us engine boot +
~5.5us ramp (wb + first tile) + 64x1.16us DVE span + ~6us tail.

Sharding: data-parallel over batch, 4 batches per core; W replicated
(broadcast to [P,H] on the host — a pure reshape/copy of the tiny W).
"""

import os
import sys

sys.path.insert(0, "/opt/trn_rl_repo")

import numpy as np
import ml_dtypes

import concourse.bass as bass
import concourse.tile as tile
from concourse import bacc, mybir
from concourse.bass_utils import run_bass_kernel_spmd

B, S, H = 32, 2048, 1024
NCORES = 8
BL = B // NCORES          # 4 batches per core
P = 128                   # SBUF partitions
T = S // P                # 16 s-tiles per batch
NB = H // P               # 8 h-blocks per tile
F32 = mybir.dt.float32
BF16 = mybir.dt.bfloat16
FP8 = mybir.dt.float8e4

EXP_G = int(os.environ.get("EXP_G", "4"))    # exp group width (columns)
ENC_BUFS = int(os.environ.get("ENC_BUFS", "40"))
# Transposed-PE score tiles per batch.  Leave at 0: each XBAR transpose
# stalls the whole 16-engine DMA fabric (~1us) and measured +22us/tile of
# end-to-end serialization — the PE score path loses to the DVE's 1.14us.
NPE = int(os.environ.get("NPE", "0"))

LAST_RESULTS = None       # test harness introspection


def _build_bass():
    nc = bacc.Bacc("TRN2", target_bir_lowering=False, debug=False)

    enc = nc.dram_tensor("enc", [BL, S, H], BF16, kind="ExternalInput").ap()
    # wenc arrives pre-broadcast to [P, H] from the host (W is tiny).
    wenc = nc.dram_tensor("wenc", [P, H], BF16, kind="ExternalInput").ap()
    wcol = nc.dram_tensor("wcol", [P, NB], BF16, kind="ExternalInput").ap()
    out = nc.dram_tensor("out", [BL, H], F32, kind="ExternalOutput").ap()

    # PE-path tiles: the last NPE columns of each batch, so the DVE exp
    # groups stay contiguous over columns 0..T-NPE-1.
    pe_set = set(range(T - NPE, T)) if NPE > 0 else set()
    ndve = T - NPE

    with tile.TileContext(nc) as tc:
        from contextlib import ExitStack

        with ExitStack() as ctx:
            wpool = ctx.enter_context(tc.tile_pool(name="wpool", bufs=1))
            encp = ctx.enter_context(tc.tile_pool(name="encp", bufs=ENC_BUFS))
            encTp = ctx.enter_context(tc.tile_pool(name="encTp", bufs=2 * max(NPE, 1)))
            scr = ctx.enter_context(tc.tile_pool(name="scr", bufs=4))
            sp = ctx.enter_context(tc.tile_pool(name="sp", bufs=2))
            # PSUM is 8 banks: psp = ps (2 banks) x 2 bufs; pstp = the batch's
            # 3 matvec chains share one [1, 3*128] bank x 2 bufs; psb per-buf
            # = ecol (1 bank) + pt (1 bank), 1 buf.
            psp = ctx.enter_context(tc.tile_pool(name="psp", bufs=2, space="PSUM"))
            pstp = ctx.enter_context(tc.tile_pool(name="pstp", bufs=2, space="PSUM"))
            psb = ctx.enter_context(tc.tile_pool(name="psb", bufs=1, space="PSUM"))

            # wb load on the scalar engine's HWDGE queue so it is not stuck
            # behind the first batch's enc DMAs on the sync queues; split by
            # PARTITIONS (32 full-width descriptors each) so four queues
            # carry it and it lands ~5us in — the first score op gates on it.
            wb = wpool.tile([P, H], BF16, name="wb")
            for i in range(4):
                nc.scalar.dma_start(
                    wb[32 * i : 32 * (i + 1), :], wenc[32 * i : 32 * (i + 1), :]
                )
            if NPE > 0:
                # NOTE: [128, 8] = 128 sixteen-byte descriptors — keep this
                # tiny DMA off the enc path and NEVER emit it when unused.
                wcolT = wpool.tile([P, NB], BF16, name="wcolT")
                nc.scalar.dma_start(wcolT[:], wcol[:])
            ones = wpool.tile([P, 1], F32, name="ones")
            nc.vector.memset(ones[:], 1.0)
            one1 = wpool.tile([1, 1], BF16, name="one1")
            nc.vector.memset(one1[:], 1.0)

            pending_tail = None  # previous batch's deferred reduction tail

            for b in range(BL):
                enc_b = enc[b].rearrange("(t p) h -> t p h", p=P)  # [T,P,H] view

                tiles = {}
                encT = {}
                for t in range(T):
                    et = encp.tile([P, H], BF16, name=f"enc_{b}_{t}", tag="enc")
                    if b == 0 and t == 0:
                        # Ramp: split tile 0 over 4 queues so scoring starts
                        # ~3x earlier (one DMA binds its descriptors to a
                        # single queue).
                        for i in range(4):
                            nc.sync.dma_start(
                                et[32 * i : 32 * (i + 1), :],
                                enc_b[t][32 * i : 32 * (i + 1), :],
                            )
                    else:
                        nc.sync.dma_start(et[:], enc_b[t])
                    tiles[t] = et
                    if t >= 4 and (t - 4) % 5 == 0 and (t - 4) // 5 < NPE:
                        # Whole-tile XBAR transpose (block-major [p, k, s]) on
                        # the scalar HWDGE queue.  A transpose spreads its
                        # descriptors over ALL 16 DMA engines (~1us fabric-wide
                        # stall), so emit at most one per ~5 enc tiles.
                        tp = sorted(pe_set)[(t - 4) // 5]
                        eT = encTp.tile(
                            [P, H], BF16, name=f"encT_{b}_{tp}", tag="encT"
                        )
                        nc.scalar.dma_start(
                            eT[:].rearrange("p (k s) -> p k s", k=NB),
                            enc_b[tp],
                            transpose=True,
                        )
                        encT[tp] = eT

                e = sp.tile([P, T], BF16, name=f"e_{b}", tag="e")
                ps = psp.tile([1, H], F32, name=f"ps_{b}", tag="ps")

                # --- PE-path matvec chains (PE only; no Scalar coupling) ---
                pst_all = (
                    pstp.tile([1, NPE * P], F32, name=f"pst_{b}", tag="pst")
                    if NPE > 0
                    else None
                )
                for i, t in enumerate(sorted(pe_set)):
                    pst = pst_all[:, i * P : (i + 1) * P]
                    for k in range(NB):
                        nc.tensor.matmul(
                            pst,
                            lhsT=wcolT[:, k : k + 1],
                            rhs=encT[t][:, k * P : (k + 1) * P],
                            start=(k == 0),
                            stop=(k == NB - 1),
                            skip_group_check=True,
                        )

                # --- DVE scores + exp groups + ws ---
                escore = sp.tile([P, T], F32, name=f"escore_{b}", tag="escore")
                for t in range(ndve):
                    stt_out = scr.tile([P, H], FP8, name=f"stt_{b}_{t}", tag="stt")
                    nc.vector.scalar_tensor_tensor(
                        out=stt_out[:],
                        in0=tiles[t][:],
                        scalar=1.0,
                        in1=wb[:],
                        op0=mybir.AluOpType.mult,
                        op1=mybir.AluOpType.mult,
                        accum_out=escore[:, t : t + 1],
                    )

                groups = []
                lo = 0
                while lo < ndve:
                    hi = min(lo + EXP_G, ndve)
                    groups.append((lo, hi))
                    lo = hi
                if b == BL - 1 and groups:
                    # last batch: per-column tail
                    lo, hi = groups.pop()
                    groups += [(t, t + 1) for t in range(lo, hi)]
                for lo, hi in groups:
                    nc.scalar.activation(
                        e[:, lo:hi], escore[:, lo:hi],
                        mybir.ActivationFunctionType.Exp,
                    )
                    for t in range(lo, hi):
                        for h0 in (0, 512):
                            nc.tensor.matmul(
                                ps[:, h0 : h0 + 512],
                                lhsT=e[:, t : t + 1],
                                rhs=tiles[t][:, h0 : h0 + 512],
                                start=(t == 0),
                                stop=(not pe_set) and (t == ndve - 1),
                            )

                # --- PE-path e-columns (Scalar work sits BEHIND the exp
                # groups so it never delays the DVE-side ws chain) ---
                for i, t in enumerate(sorted(pe_set)):
                    erow = sp.tile([1, P], BF16, name=f"erow_{b}_{t}", tag="erow")
                    nc.scalar.activation(
                        erow[:], pst_all[:, i * P : (i + 1) * P],
                        mybir.ActivationFunctionType.Exp,
                    )
                    ecol_ps = psb.tile([P, 1], F32, name=f"ecol_{b}_{t}", tag="ecol")
                    nc.tensor.matmul(
                        ecol_ps[:], lhsT=erow[:], rhs=one1[:], start=True, stop=True
                    )
                    nc.scalar.copy(e[:, t : t + 1], ecol_ps[:])

                # PE-path ws matmuls close the chain.
                last_pe = max(pe_set) if pe_set else None
                for t in sorted(pe_set):
                    for h0 in (0, 512):
                        nc.tensor.matmul(
                            ps[:, h0 : h0 + 512],
                            lhsT=e[:, t : t + 1],
                            rhs=tiles[t][:, h0 : h0 + 512],
                            start=False,
                            stop=(t == last_pe),
                        )

                # Defer this batch's reduction tail until after the NEXT
                # batch's score ops are emitted: the esum reduce + reciprocal
                # otherwise sit in the DVE's in-order stream at each batch
                # boundary, idling it ~1.5us on a Scalar->DVE->PE->DVE
                # semaphore round-trip.  Deferred, the reduce lands mid-next-
                # batch when its inputs are long since ready.
                def _tail(b=b, e=e, ps=ps):
                    esum = sp.tile([P, 1], F32, name=f"esum_{b}", tag="esum")
                    nc.vector.tensor_reduce(
                        esum[:], e[:], axis=mybir.AxisListType.X,
                        op=mybir.AluOpType.add,
                    )
                    pt = psb.tile([1, 1], F32, name=f"pt_{b}", tag="pt")
                    nc.tensor.matmul(
                        pt[:], lhsT=ones[:], rhs=esum[:], start=True, stop=True
                    )
                    rtot = sp.tile([1, 1], F32, name=f"rtot_{b}", tag="rtot")
                    nc.vector.reciprocal(rtot[:], pt[:])
                    ob = sp.tile([1, H], F32, name=f"ob_{b}", tag="ob")
                    for h0 in (0, 512):
                        nc.scalar.mul(
                            ob[:, h0 : h0 + 512], ps[:, h0 : h0 + 512], rtot[:]
                        )
                        nc.sync.dma_start(
                            out[b : b + 1, h0 : h0 + 512], ob[:, h0 : h0 + 512]
                        )

                if b == BL - 1:
                    # last batch: no next batch to hide behind; emit now
                    if pending_tail is not None:
                        pending_tail()
                    _tail()
                else:
                    if pending_tail is not None:
                        pending_tail()
                    pending_tail = _tail

    nc.compile()
    return nc


_NC_CACHE = None


def kernel(decoder_hidden, encoder_hidden_outputs, W, b):
    global _NC_CACHE, LAST_RESULTS
    enc_full = np.ascontiguousarray(
        np.asarray(encoder_hidden_outputs, dtype=np.float32).astype(ml_dtypes.bfloat16)
    )
    w_enc16 = np.asarray(W, dtype=np.float32)[H:, 0].astype(ml_dtypes.bfloat16)
    w_bcast = np.ascontiguousarray(np.broadcast_to(w_enc16, (P, H)))
    w_col = np.ascontiguousarray(w_enc16.reshape(NB, P).T)

    if _NC_CACHE is None:
        _NC_CACHE = _build_bass()
    nc = _NC_CACHE

    in_maps = [
        {"enc": enc_full[i * BL : (i + 1) * BL], "wenc": w_bcast, "wcol": w_col}
        for i in range(NCORES)
    ]
    res = run_bass_kernel_spmd(
        nc,
        in_maps,
        core_ids=list(range(NCORES)),
        trace=bool(int(os.environ.get("KERNEL_TRACE", "0"))),
    )
    LAST_RESULTS = res
    out = np.concatenate([res.results[i]["out"] for i in range(NCORES)], axis=0)
    return out.astype(np.float32)
